# revision 6
# baseline (speedup 1.0000x reference)
"""Bass/Trainium2 kernel for batched GNN message passing:
    out[b, d, n] = sum_m adj[b, n, m] * x[b, d, m]
B=2, D=3072, N=8192, fp32.

Sharding: 8 cores, core c -> (b = c//4, n-quarter = c%4). Each core computes
C[d, n_quarter] = X[b] @ A[b, n_quarter, :].T  with D=3072, NC=2048, M=8192.
Zero collectives; host slices inputs and concatenates outputs.

Per-core kernel: K-split into KQ passes. Per K-pass, the full-width adjT panel
[128, KC, NC] is built once via TensorE 128x128 transposes, then a d-block loop
transposes an X strip and runs fp32r matmuls (1 cyc/row) accumulating 4 PSUM
banks over the n-width. Partial outputs accumulate in DRAM across K-passes.
"""

import sys
from contextlib import ExitStack

import numpy as np

sys.path.insert(0, "/opt/trn_rl_repo")

B = 2
D = 3072
N = 8192
NCORES = 8
NSPLIT = 4  # n-quarters per batch sample
NC = N // NSPLIT  # 2048 columns of out per core


def build_program(d=D, ncols=NC, m=N, kq=8, nbw=512):
    """Build the per-core Bass program. Returns compiled nc."""
    import concourse.mybir as mybir
    import concourse.tile as tile
    from concourse import bacc
    from concourse.masks import make_identity

    f32 = mybir.dt.float32
    f32r = mybir.dt.float32r
    P = 128

    mq = m // kq          # contraction elems per K-pass
    kc_n = mq // P        # 128-chunks per K-pass
    n128 = ncols // P     # 128-row blocks of adj slice
    ndb = d // P          # d-blocks
    nnb = ncols // nbw    # psum banks swept per d-block

    nc = bacc.Bacc(None, target_bir_lowering=False, debug=False)

    x_ext = nc.dram_tensor("x", [d, m], f32r, kind="ExternalInput")
    a_ext = nc.dram_tensor("adj", [ncols, m], f32r, kind="ExternalInput")
    out_ext = nc.dram_tensor("out", [d, ncols], f32, kind="ExternalOutput")

    with tile.TileContext(nc) as tc, ExitStack() as ctx:
        dram = ctx.enter_context(tc.tile_pool(name="dram", bufs=1, space="DRAM"))
        c_accum = None
        if kq > 1:
            c_accum = dram.tile([d, ncols], f32, name="c_accum")

        const = ctx.enter_context(tc.tile_pool(name="const", bufs=1))
        ident_f32 = const.tile([P, P], f32)
        make_identity(nc, ident_f32)
        ident = const.tile([P, P], f32r, name="ident_r")
        nc.vector.tensor_copy(out=ident[:], in_=ident_f32[:])

        panel_pool = ctx.enter_context(tc.tile_pool(name="panel", bufs=2))
        xt_pool = ctx.enter_context(tc.tile_pool(name="xt", bufs=2))
        stg_pool = ctx.enter_context(tc.tile_pool(name="stg", bufs=3))
        out_pool = ctx.enter_context(tc.tile_pool(name="outp", bufs=4))
        cprev_pool = ctx.enter_context(tc.tile_pool(name="cprev", bufs=4))
        tp_psum = ctx.enter_context(tc.tile_pool(name="tpp", bufs=2, space="PSUM"))
        acc_psum = ctx.enter_context(tc.tile_pool(name="accp", bufs=6, space="PSUM"))

        for kqi in range(kq):
            mlo = kqi * mq
            # ---- build adjT panel [P, kc_n, ncols] for this K-pass ----
            adjt = panel_pool.tile([P, kc_n, ncols], f32r, tag="adjt")
            for nb in range(n128):
                stg_a = stg_pool.tile([P, mq], f32r, tag="stg_a")
                nc.sync.dma_start(
                    out=stg_a[:], in_=a_ext[nb * P : (nb + 1) * P, mlo : mlo + mq]
                )
                for kc in range(kc_n):
                    pt = tp_psum.tile([P, P], f32r, tag="tp")
                    nc.tensor.transpose(pt[:], stg_a[:, kc * P : (kc + 1) * P], ident[:])
                    nc.vector.tensor_copy(
                        out=adjt[:, kc, nb * P : (nb + 1) * P], in_=pt[:]
                    )

            # ---- d-block loop: transpose X strip, matmul, evict ----
            for db in range(ndb):
                stg_x = stg_pool.tile([P, mq], f32r, tag="stg_x")
                nc.sync.dma_start(
                    out=stg_x[:], in_=x_ext[db * P : (db + 1) * P, mlo : mlo + mq]
                )
                xt = xt_pool.tile([P, kc_n, P], f32r, tag="xt")
                for kc in range(kc_n):
                    pt = tp_psum.tile([P, P], f32r, tag="tp")
                    nc.tensor.transpose(pt[:], stg_x[:, kc * P : (kc + 1) * P], ident[:])
                    nc.vector.tensor_copy(out=xt[:, kc, :], in_=pt[:])

                accs = [
                    acc_psum.tile([P, nbw], f32, tag="acc", name=f"acc{i}")
                    for i in range(nnb)
                ]
                for kc in range(kc_n):
                    for nb in range(nnb):
                        nc.tensor.matmul(
                            accs[nb][:],
                            xt[:, kc, :],
                            adjt[:, kc, nb * nbw : (nb + 1) * nbw],
                            start=(kc == 0),
                            stop=(kc == kc_n - 1),
                        )

                dst = out_ext if kqi == kq - 1 else c_accum
                for nb in range(nnb):
                    osb = out_pool.tile([P, nbw], f32, tag="osb")
                    if kqi == 0:
                        nc.vector.tensor_copy(out=osb[:], in_=accs[nb][:])
                    else:
                        cprev = cprev_pool.tile([P, nbw], f32, tag="cprev")
                        nc.sync.dma_start(
                            out=cprev[:],
                            in_=c_accum[
                                db * P : (db + 1) * P, nb * nbw : (nb + 1) * nbw
                            ],
                        )
                        nc.vector.tensor_tensor(
                            out=osb[:],
                            in0=accs[nb][:],
                            in1=cprev[:],
                            op=mybir.AluOpType.add,
                        )
                    nc.sync.dma_start(
                        out=dst[db * P : (db + 1) * P, nb * nbw : (nb + 1) * nbw],
                        in_=osb[:],
                    )

    nc.compile()
    return nc


_NC_CACHE = {}


def _get_program(**kw):
    key = tuple(sorted(kw.items()))
    if key not in _NC_CACHE:
        _NC_CACHE[key] = build_program(**kw)
    return _NC_CACHE[key]


def kernel(x: np.ndarray, adj: np.ndarray) -> np.ndarray:
    """Full inputs in, full output out. x [B,D,N] f32, adj [B,N,N] f32."""
    from concourse.bass_utils import run_bass_kernel_spmd

    assert x.shape == (B, D, N) and adj.shape == (B, N, N)
    nc = _get_program()

    in_maps = []
    for c in range(NCORES):
        b, ns = divmod(c, NSPLIT)
        in_maps.append(
            {
                "x": np.ascontiguousarray(x[b], dtype=np.float32),
                "adj": np.ascontiguousarray(
                    adj[b, ns * NC : (ns + 1) * NC, :], dtype=np.float32
                ),
            }
        )

    res = run_bass_kernel_spmd(nc, in_maps, core_ids=list(range(NCORES)))
    out = np.empty((B, D, N), dtype=np.float32)
    for c in range(NCORES):
        b, ns = divmod(c, NSPLIT)
        out[b, :, ns * NC : (ns + 1) * NC] = res.results[c]["out"]
    return out


# revision 7
# speedup vs baseline: 1.1251x; 1.1251x over previous
"""Bass/Trainium2 kernel for batched GNN message passing:
    out[b, d, n] = sum_m adj[b, n, m] * x[b, d, m]
B=2, D=3072, N=8192, fp32.

Sharding: 8 cores, core c -> (b = c//4, n-quarter = c%4). Each core computes
C[d, n_quarter] = X[b] @ A[b, n_quarter, :].T  with D=3072, NC=2048, M=8192.
Zero collectives; host slices inputs and concatenates outputs.

Per-core kernel: K-split into KQ passes. Per K-pass, the full-width adjT panel
[128, KC, NC] is built once via TensorE 128x128 transposes, then a d-block loop
transposes an X strip and runs fp32r matmuls (1 cyc/row) accumulating 4 PSUM
banks over the n-width. Partial outputs accumulate in DRAM across K-passes.
"""

import sys
from contextlib import ExitStack

import numpy as np

sys.path.insert(0, "/opt/trn_rl_repo")

B = 2
D = 3072
N = 8192
NCORES = 8
NSPLIT = 4  # n-quarters per batch sample
NC = N // NSPLIT  # 2048 columns of out per core


def build_program(d=D, ncols=NC, m=N, kq=4, nbw=512):
    """Build the per-core Bass program. Returns compiled nc."""
    import concourse.mybir as mybir
    import concourse.tile as tile
    from concourse import bacc
    from concourse.masks import make_identity

    f32 = mybir.dt.float32
    f32r = mybir.dt.float32r
    P = 128

    mq = m // kq          # contraction elems per K-pass
    kc_n = mq // P        # 128-chunks per K-pass
    n128 = ncols // P     # 128-row blocks of adj slice
    ndb = d // P          # d-blocks
    nnb = ncols // nbw    # psum banks swept per d-block

    nc = bacc.Bacc(None, target_bir_lowering=False, debug=False)

    x_ext = nc.dram_tensor("x", [d, m], f32r, kind="ExternalInput")
    a_ext = nc.dram_tensor("adj", [ncols, m], f32r, kind="ExternalInput")
    out_ext = nc.dram_tensor("out", [d, ncols], f32, kind="ExternalOutput")

    with tile.TileContext(nc) as tc, ExitStack() as ctx:
        dram = ctx.enter_context(tc.tile_pool(name="dram", bufs=1, space="DRAM"))
        c_accum = None
        if kq > 1:
            c_accum = dram.tile([d, ncols], f32, name="c_accum")

        const = ctx.enter_context(tc.tile_pool(name="const", bufs=1))
        ident_f32 = const.tile([P, P], f32)
        make_identity(nc, ident_f32)
        ident = const.tile([P, P], f32r, name="ident_r")
        nc.vector.tensor_copy(out=ident[:], in_=ident_f32[:])

        panel_pool = ctx.enter_context(tc.tile_pool(name="panel", bufs=1))
        xt_pool = ctx.enter_context(tc.tile_pool(name="xt", bufs=2))
        stg_pool = ctx.enter_context(tc.tile_pool(name="stg", bufs=2))
        out_pool = ctx.enter_context(tc.tile_pool(name="outp", bufs=3))
        cprev_pool = ctx.enter_context(tc.tile_pool(name="cprev", bufs=3))
        tp_psum = ctx.enter_context(tc.tile_pool(name="tpp", bufs=2, space="PSUM"))
        acc_psum = ctx.enter_context(tc.tile_pool(name="accp", bufs=6, space="PSUM"))

        for kqi in range(kq):
            mlo = kqi * mq
            # ---- build adjT panel [P, kc_n, ncols] for this K-pass ----
            adjt = panel_pool.tile([P, kc_n, ncols], f32r, tag="adjt")
            for nb in range(n128):
                stg_a = stg_pool.tile([P, mq], f32r, tag="stg_a")
                nc.sync.dma_start(
                    out=stg_a[:], in_=a_ext[nb * P : (nb + 1) * P, mlo : mlo + mq]
                )
                for kc in range(kc_n):
                    pt = tp_psum.tile([P, P], f32r, tag="tp")
                    nc.tensor.transpose(pt[:], stg_a[:, kc * P : (kc + 1) * P], ident[:])
                    nc.vector.tensor_copy(
                        out=adjt[:, kc, nb * P : (nb + 1) * P], in_=pt[:]
                    )

            # ---- d-block loop: transpose X strip, matmul, evict ----
            for db in range(ndb):
                stg_x = stg_pool.tile([P, mq], f32r, tag="stg_x")
                nc.sync.dma_start(
                    out=stg_x[:], in_=x_ext[db * P : (db + 1) * P, mlo : mlo + mq]
                )
                xt = xt_pool.tile([P, kc_n, P], f32r, tag="xt")
                for kc in range(kc_n):
                    pt = tp_psum.tile([P, P], f32r, tag="tp")
                    nc.tensor.transpose(pt[:], stg_x[:, kc * P : (kc + 1) * P], ident[:])
                    nc.vector.tensor_copy(out=xt[:, kc, :], in_=pt[:])

                accs = [
                    acc_psum.tile([P, nbw], f32, tag="acc", name=f"acc{i}")
                    for i in range(nnb)
                ]
                for kc in range(kc_n):
                    for nb in range(nnb):
                        nc.tensor.matmul(
                            accs[nb][:],
                            xt[:, kc, :],
                            adjt[:, kc, nb * nbw : (nb + 1) * nbw],
                            start=(kc == 0),
                            stop=(kc == kc_n - 1),
                        )

                dst = out_ext if kqi == kq - 1 else c_accum
                for nb in range(nnb):
                    osb = out_pool.tile([P, nbw], f32, tag="osb")
                    if kqi == 0:
                        nc.vector.tensor_copy(out=osb[:], in_=accs[nb][:])
                    else:
                        cprev = cprev_pool.tile([P, nbw], f32, tag="cprev")
                        nc.sync.dma_start(
                            out=cprev[:],
                            in_=c_accum[
                                db * P : (db + 1) * P, nb * nbw : (nb + 1) * nbw
                            ],
                        )
                        nc.vector.tensor_tensor(
                            out=osb[:],
                            in0=accs[nb][:],
                            in1=cprev[:],
                            op=mybir.AluOpType.add,
                        )
                    nc.sync.dma_start(
                        out=dst[db * P : (db + 1) * P, nb * nbw : (nb + 1) * nbw],
                        in_=osb[:],
                    )

    nc.compile()
    return nc


_NC_CACHE = {}


def _get_program(**kw):
    key = tuple(sorted(kw.items()))
    if key not in _NC_CACHE:
        _NC_CACHE[key] = build_program(**kw)
    return _NC_CACHE[key]


def kernel(x: np.ndarray, adj: np.ndarray) -> np.ndarray:
    """Full inputs in, full output out. x [B,D,N] f32, adj [B,N,N] f32."""
    from concourse.bass_utils import run_bass_kernel_spmd

    assert x.shape == (B, D, N) and adj.shape == (B, N, N)
    nc = _get_program()

    in_maps = []
    for c in range(NCORES):
        b, ns = divmod(c, NSPLIT)
        in_maps.append(
            {
                "x": np.ascontiguousarray(x[b], dtype=np.float32),
                "adj": np.ascontiguousarray(
                    adj[b, ns * NC : (ns + 1) * NC, :], dtype=np.float32
                ),
            }
        )

    res = run_bass_kernel_spmd(nc, in_maps, core_ids=list(range(NCORES)))
    out = np.empty((B, D, N), dtype=np.float32)
    for c in range(NCORES):
        b, ns = divmod(c, NSPLIT)
        out[b, :, ns * NC : (ns + 1) * NC] = res.results[c]["out"]
    return out
